# revision 1
# baseline (speedup 1.0000x reference)
"""Causal linear attention (B=2, H=8, T=2048, D=64) on 8 Trainium2 NeuronCores.

Sharding: the 16 (batch, head) pairs are split 2-per-core (pure data/head
parallelism; the per-(b,h) recurrence is independent so no collectives).

Per (b,h) the kernel runs a chunked scan over T in chunks of C=128:
  out_chunk = tril(Qp Kp^T) @ [V|1]  +  Qp @ S ,   S += Kp^T @ [V|1]
where Qp/Kp = elu(.)+1 feature maps and the appended ones-column of V
produces the normalizer z in column D of the output accumulation.

Host-side prep: q/k/v are pre-transposed on CPU into the on-chip layout
[128 partitions, chunk, (src,) head, d] and pre-cast to bf16, so every DMA
is one contiguous descriptor per partition (q and k are interleaved into a
single DRAM tensor so one DMA + one activation covers both; v gets its
ones-column appended on the host).

Both heads ride through shared ops: one PE transpose pair per chunk yields
Qp^T/Kp^T for both heads (head h on partitions [64h, 64h+64)).  Per-head
matmul outputs go to separate PSUM banks (two matmuls writing disjoint
column ranges of ONE bank crash the device - probed), but each pair of
banks is allocated as a single 2-bank tile so one Vector/Scalar instruction
evacuates both heads, halving the per-op overhead.
"""

import sys

sys.path.insert(0, "/opt/trn_rl_repo")

from contextlib import ExitStack

import numpy as np
import ml_dtypes

import concourse.bass as bass
import concourse.bacc as bacc
import concourse.mybir as mybir
import concourse.tile as tile
from concourse.bass_utils import run_bass_kernel_spmd

B, H, T, D = 2, 8, 2048, 64
N_CORES = 8
PAIRS = B * H                  # 16 (batch, head) pairs
PPC = PAIRS // N_CORES         # 2 pairs per core
C = 128                        # chunk (= partition) size
NCH = T // C                   # 16 chunks
DV = D + 1                     # value dim incl normalizer ones-column
BANK_F32 = 512                 # fp32 slots per 2 KiB PSUM bank

F32 = mybir.dt.float32
BF16 = mybir.dt.bfloat16
AF = mybir.ActivationFunctionType
ALU = mybir.AluOpType

BF = ml_dtypes.bfloat16

# DMA load segments (chunk ranges) and feature-map segments.
LOAD_SEGS = [(0, 1), (1, 2), (2, 3), (3, 5), (5, 10), (10, 16)]
FM_SEGS = [(0, 1), (1, 2), (2, 3), (3, 5), (5, 10), (10, 16)]
# normalize+store batches: chunk at which each fires -> chunk range
NORM_BATCHES = {7: (0, 8), 11: (8, 12), 13: (12, 14), 14: (14, 15), 15: (15, 16)}

_CACHE = {}


def _build():
    nc = bacc.Bacc(None, target_bir_lowering=False)
    qkv_d = nc.dram_tensor(
        "qkv", [C, NCH, 2 * PPC * D + PPC * DV], BF16, kind="ExternalInput"
    )
    o_d = nc.dram_tensor("out", [C, NCH, PPC, D], F32, kind="ExternalOutput")

    with ExitStack() as ctx:
        tc = ctx.enter_context(tile.TileContext(nc))
        consts = ctx.enter_context(tc.tile_pool(name="consts", bufs=1))
        loads = ctx.enter_context(tc.tile_pool(name="loads", bufs=1))
        fmp = ctx.enter_context(tc.tile_pool(name="fmp", bufs=1))
        tpose = ctx.enter_context(tc.tile_pool(name="tpose", bufs=3))
        ampool = ctx.enter_context(tc.tile_pool(name="ampool", bufs=2))
        spool = ctx.enter_context(tc.tile_pool(name="spool", bufs=3))
        opool = ctx.enter_context(tc.tile_pool(name="opool", bufs=1))
        finpool = ctx.enter_context(tc.tile_pool(name="finpool", bufs=2))
        ps_t = ctx.enter_context(tc.tile_pool(name="ps_t", bufs=3, space="PSUM"))
        ps_a = ctx.enter_context(tc.tile_pool(name="ps_a", bufs=2, space="PSUM"))
        ps_s = ctx.enter_context(tc.tile_pool(name="ps_s", bufs=1, space="PSUM"))

        ident_d = nc.inline_tensor(
            np.eye(C, dtype=np.float32).astype(BF), name="ident_c"
        )
        # A^T[s,t] keeps s<=t: upper-triangular mask replicated per head (bf16).
        mask_d = nc.inline_tensor(
            np.ascontiguousarray(
                np.broadcast_to(
                    np.triu(np.ones((C, C), np.float32))[:, None, :], (C, PPC, C)
                )
            ).astype(BF),
            name="mask_c",
        )
        ident = consts.tile([C, C], BF16, tag="ident")
        mask = consts.tile([C, PPC, C], BF16, tag="mask")

        # Raw loads and feature-mapped working set, head-interleaved.
        # Flat 2D tiles: engine ops stream fast only on low-rank APs.
        QKW = 2 * PPC * D          # qk columns per chunk (256)
        VW = PPC * DV              # v columns per chunk (130)
        CW = QKW + VW              # total columns per chunk (386)
        qkvf = loads.tile([C, NCH * CW], BF16, tag="qkvf", name="qkvf")
        qkp = loads.tile([C, NCH * QKW], BF16, tag="qkp", name="qkp")
        osb = opool.tile([C, NCH, PPC, DV], BF16, tag="osb", name="osb")

        def emit_load(s0, s1, qk_eng=None):
            (qk_eng or nc.sync).dma_start(
                out=qkvf[:, s0 * CW : s1 * CW], in_=qkv_d[:, s0:s1]
            )

        def emit_fm(s0, s1):
            nseg = s1 - s0
            ncols = nseg * QKW
            src_ap = bass.AP(
                tensor=qkvf.tensor,
                offset=qkvf.offset + s0 * CW,
                ap=[qkvf.ap[0], [CW, nseg], [1, QKW]],
            )
            cols = slice(s0 * QKW, s1 * QKW)
            # feature map: elu(x)+1 == max(min(exp(x), 1), x+1)
            e = fmp.tile([C, 6 * QKW], BF16, tag="e", name="e", bufs=2)
            nc.scalar.activation(out=e[:, :ncols], in_=src_ap, func=AF.Exp)
            a = fmp.tile([C, 6 * QKW], BF16, tag="a", name="a", bufs=2)
            nc.vector.tensor_scalar_add(
                out=a[:, :ncols], in0=src_ap, scalar1=1.0
            )
            nc.vector.scalar_tensor_tensor(
                out=qkp[:, cols],
                in0=e[:, :ncols],
                scalar=1.0,
                in1=a[:, :ncols],
                op0=ALU.min,
                op1=ALU.max,
            )

        emit_load(*LOAD_SEGS[0])
        nc.scalar.dma_start(out=ident, in_=ident_d[:, :])
        nc.scalar.dma_start(out=mask, in_=mask_d[:, :])
        emit_load(*LOAD_SEGS[1])
        emit_fm(*FM_SEGS[0])
        emit_load(*LOAD_SEGS[2])
        emit_fm(*FM_SEGS[1])


        # Running state in PSUM: head h accumulates on partitions [64h, 64h+64).
        s_psum = ps_s.tile([C, 512], F32, tag="s", name="s_psum")
        s_stash = {}

        for n in range(NCH):
            if n == 1:
                emit_fm(*FM_SEGS[2])
            elif n == 2:
                emit_load(*LOAD_SEGS[3])
                emit_fm(*FM_SEGS[3])
            elif n == 3:
                emit_load(*LOAD_SEGS[4])
            elif n == 4:
                emit_fm(*FM_SEGS[4])
            elif n == 5:
                emit_load(*LOAD_SEGS[5])
            elif n == 7:
                emit_fm(*FM_SEGS[5])

            # PE transposes: [128t, (2h x 64d)] -> partitions (h,d), free t.
            # Both land in one bank (transpose-mode matmuls tolerate this).
            tT = ps_t.tile([C, 2, C], BF16, tag="tT", name="tT")
            nc.tensor.transpose(tT[:, 0, :], qkp[:, n * QKW : n * QKW + C], ident)
            nc.tensor.transpose(
                tT[:, 1, :], qkp[:, n * QKW + C : n * QKW + 2 * C], ident
            )

            # State update first so the snapshot is ready a chunk early.
            if n < NCH - 1:
                for h in range(PPC):
                    lo = h * D
                    nc.tensor.matmul(
                        s_psum[lo : lo + D, 0:DV],
                        qkp[:, n * QKW + C + lo : n * QKW + C + lo + D],
                        qkvf[:, n * CW + QKW + h * DV : n * CW + QKW + (h + 1) * DV],
                        start=(n == 0),
                        stop=True,
                        tile_position=(0, lo),
                        skip_group_check=True,
                    )
                s_new = spool.tile([C, DV], BF16, tag="sb", name="sb")
                nc.scalar.activation(out=s_new, in_=s_psum[:, 0:DV], func=AF.Copy)
                s_stash[n] = s_new

            qkT = tpose.tile([C, 2, C], BF16, tag="qkT", name="qkT")
            if n % 2 == 0:
                nc.vector.tensor_copy(out=qkT, in_=tT)
            else:
                nc.scalar.activation(out=qkT, in_=tT, func=AF.Copy)

            # A^T = Kp Qp^T per head, each head in its own bank of a pair tile.
            at = ps_a.tile([C, PPC, BANK_F32], F32, tag="at", name="at")
            for h in range(PPC):
                lo, hi = h * D, (h + 1) * D
                nc.tensor.matmul(
                    at[:, h, 0:C],
                    qkT[lo:hi, 1, :],
                    qkT[lo:hi, 0, :],
                    start=True,
                    stop=True,
                    skip_group_check=True,
                )
            am = ampool.tile([C, PPC, C], BF16, tag="am", name="am")
            nc.vector.tensor_tensor(am, at[:, :, 0:C], mask, op=ALU.mult)

            # Output accumulation, per-head banks of a pair tile.
            # Output accumulation reuses the at pair banks (at is dead once
            # the mask op has read it; same-column overwrite with start=True).
            op_ = at
            for h in range(PPC):
                lo, hi = h * D, (h + 1) * D
                if n > 0:
                    # inter-chunk term first: ready before the mask lands
                    nc.tensor.matmul(
                        op_[:, h, 0:DV],
                        qkT[lo:hi, 0, :],
                        s_stash[n - 1][lo:hi, :],
                        start=True,
                        stop=False,
                        skip_group_check=True,
                    )
                nc.tensor.matmul(
                    op_[:, h, 0:DV],
                    am[:, h, :],
                    qkvf[:, n * CW + QKW + h * DV : n * CW + QKW + (h + 1) * DV],
                    start=(n == 0),
                    stop=True,
                    skip_group_check=True,
                )
            nc.scalar.activation(
                out=osb[:, n, :, :], in_=op_[:, :, 0:DV], func=AF.Copy
            )

            batch = NORM_BATCHES.get(n)
            if batch is not None:
                b0, b1 = batch
                nb = b1 - b0
                rows = slice(b0, b1)
                rz = finpool.tile([C, 8, PPC], F32, tag="rz", name="rz", bufs=4)
                nc.vector.reciprocal(out=rz[:, :nb, :], in_=osb[:, rows, :, D])
                fin = finpool.tile([C, 8, PPC, D], F32, tag="fin", name="fin", bufs=2)
                rz_b = bass.AP(
                    tensor=rz.tensor,
                    offset=rz.offset,
                    ap=[rz.ap[0], [rz.ap[1][0], nb], rz.ap[2], [0, D]],
                )
                eng = nc.gpsimd if nb >= 4 else nc.vector
                eng.tensor_tensor(
                    out=fin[:, :nb, :, :],
                    in0=osb[:, rows, :, 0:D],
                    in1=rz_b,
                    op=ALU.mult,
                )
                nc.sync.dma_start(out=o_d[:, rows], in_=fin[:, :nb, :, :])

    nc.compile()
    return nc


def _get_program():
    if "nc" not in _CACHE:
        _CACHE["nc"] = _build()
    return _CACHE["nc"]


def _prep(x):
    # [B,H,T,D] -> per-core [C, NCH, PPC, D] bf16 arrays
    xr = np.asarray(x, np.float32).reshape(PAIRS, NCH, C, D)
    xr = np.ascontiguousarray(xr.transpose(2, 1, 0, 3)).astype(BF)  # [C,NCH,PAIRS,D]
    return [xr[:, :, i * PPC : (i + 1) * PPC, :] for i in range(N_CORES)]


def run_sharded(q, k, v, trace=False, **kwargs):
    """Run on 8 cores; returns (full_output, BassKernelResults)."""
    nc = _get_program()
    qs = _prep(q)
    ks = _prep(k)
    vs = _prep(v)
    ones = np.ones((C, NCH, PPC, 1), np.float32).astype(BF)
    in_maps = []
    for i in range(N_CORES):
        qk = np.stack([qs[i], ks[i]], axis=2).reshape(C, NCH, 2 * PPC * D)
        vb = np.concatenate([vs[i], ones], axis=3).reshape(C, NCH, PPC * DV)
        qkv = np.ascontiguousarray(np.concatenate([qk, vb], axis=2))
        in_maps.append({"qkv": qkv})
    res = run_bass_kernel_spmd(
        nc, in_maps, core_ids=list(range(N_CORES)), trace=trace, **kwargs
    )
    # out per core: [C, NCH, PPC, D] fp32 -> [B,H,T,D]
    out = np.concatenate(
        [res.results[i]["out"] for i in range(N_CORES)], axis=2
    )  # [C, NCH, PAIRS, D]
    out = out.transpose(2, 1, 0, 3).reshape(B, H, T, D)
    return np.ascontiguousarray(out, dtype=np.float32), res


def kernel(q, k, v):
    out, _ = run_sharded(q, k, v)
    return out

